# revision 19
# baseline (speedup 1.0000x reference)
"""Fused attention kernel for Trainium2, 8 NeuronCores.

Problem: B=4, T=2048, C=1024, nh=16, hs=64, fused QKV (chunk order k,q,v),
softmax attention, then (faithful reference bug) reshape (B,nh,T,hs)->(B,T,C)
directly before the output projection.

Key structural fact: with the buggy reshape, head h's attention output
occupies exactly rows [h*128, (h+1)*128) of the reshaped (T, C) matrix
(row tau = h*128 + t//16, col = (t%16)*64 + d). So everything after the
QKV projection is fully independent per (batch, head) pair; the output
projection needs no cross-head reduction.

Sharding: 8 cores = 4 batches x 2 head-groups (8 heads each). Each core
computes its batch's QKV slice and its 8 heads end-to-end. No collectives.
"""

import sys

import numpy as np

sys.path.insert(0, "/opt/trn_rl_repo")

import ml_dtypes  # noqa: E402

B, T, C = 4, 2048, 1024
NH, HS = 16, 64
NCORES = 8
HPC = 8  # heads per core

_CACHE = {}


def _build():
    from contextlib import ExitStack

    import concourse.bass as bass  # noqa: F401
    import concourse.mybir as mybir
    from concourse import bacc, tile

    F32 = mybir.dt.float32
    F32R = mybir.dt.float32r
    BF16 = mybir.dt.bfloat16
    ADD = mybir.AluOpType.add
    MULT = mybir.AluOpType.mult
    EXP = mybir.ActivationFunctionType.Exp

    nc = bacc.Bacc()
    xT = nc.dram_tensor("xT", [128, 8, 2048], F32R, kind="ExternalInput")
    wqkv = nc.dram_tensor("wqkv", [128, 8, 1536], F32R, kind="ExternalInput")
    bqk = nc.dram_tensor("bqk", [128, 8], F32, kind="ExternalInput")
    bv = nc.dram_tensor("bv", [128, 512], F32, kind="ExternalInput")
    wp = nc.dram_tensor("wp", [64, 16, 1024], BF16, kind="ExternalInput")
    pb = nc.dram_tensor("pb", [128, 1024], F32, kind="ExternalInput")
    vones = nc.dram_tensor("vones", [128, 1], BF16, kind="ExternalInput")
    ones1 = nc.dram_tensor("ones1", [1, 64], F32R, kind="ExternalInput")
    y = nc.dram_tensor("y", [128, 8, 1024], F32, kind="ExternalOutput")

    with tile.TileContext(nc) as tc, ExitStack() as ctx:
        persist = ctx.enter_context(tc.tile_pool(name="persist", bufs=1))
        yps = ctx.enter_context(tc.tile_pool(name="ysb", bufs=2))

        scratch = persist.tile([128, 4], F32, tag="scratch")
        bqk_sb = persist.tile([128, 8], F32, tag="bqk")
        nc.sync.dma_start(bqk_sb, bqk[:])
        nc.vector.tensor_copy(scratch[:, 0:1], bqk_sb[:, 0:1])
        pb_sb = persist.tile([128, 1024], F32, tag="pb")
        nc.sync.dma_start(pb_sb, pb[:])
        nc.vector.tensor_copy(scratch[:, 1:2], pb_sb[:, 0:1])
        ones1_sb = persist.tile([1, 64], F32R, tag="ones1")
        nc.sync.dma_start(ones1_sb, ones1[:])
        vbuf = persist.tile([128, 16, HPC, 64], BF16, tag="vbuf")
        ones128_sb = persist.tile([128, 1], BF16, tag="ones128")
        nc.sync.dma_start(ones128_sb, vones[:])
        nc.vector.tensor_copy(scratch[:, 2:3], ones128_sb)
        # K^T rows in tiles 0-3 (d on partitions, t free), Q^T rows in 4-7
        qk = [persist.tile([128, 2048], F32R, tag=f"qk{mt}", name=f"qk{mt}")
              for mt in range(8)]

        # ---------------- QKV phase ----------------
        with tc.tile_pool(name="qkvsb", bufs=1) as qsb, \
             tc.tile_pool(name="wstream", bufs=2) as wsp, \
             tc.tile_pool(name="qkvps", bufs=2, space="PSUM") as qps:
            bv_sb = qsb.tile([128, 512], F32, tag="bv")
            nc.sync.dma_start(bv_sb, bv[:])
            nc.vector.tensor_copy(scratch[:, 3:4], bv_sb[:, 0:1])
            xts = [qsb.tile([128, 2048], F32R, tag=f"xt{ct}", name=f"xt{ct}")
                   for ct in range(8)]
            for q in range(4):
                for ct in range(8):
                    nc.sync.dma_start(xts[ct][:, q * 512:(q + 1) * 512],
                                      xT[:, ct, q * 512:(q + 1) * 512])

            # icx-outer: the first psum group only needs the first quarter
            # of xT (2 MB), so matmuls start ~10us in instead of ~50us
            for icx in range(4):
                for mt in range(8):
                    wt = wsp.tile([128, 8, 128], F32R, tag="wt", name="wt")
                    nc.sync.dma_start(wt, wqkv[:, :, mt * 128:(mt + 1) * 128])
                    ps = qps.tile([128, 512], F32, tag="qkvps")
                    for ct in range(8):
                        nc.tensor.matmul(
                            ps, wt[:, ct, :], xts[ct][:, icx * 512:(icx + 1) * 512],
                            start=(ct == 0), stop=(ct == 7))
                    nc.vector.tensor_tensor(
                        qk[mt][:, icx * 512:(icx + 1) * 512], ps,
                        bqk_sb[:, mt:mt + 1].to_broadcast((128, 512)), ADD)

            wv_sb = qsb.tile([128, 8, 512], F32R, tag="wv")
            nc.sync.dma_start(wv_sb, wqkv[:, :, 1024:1536])
            for tt in range(16):
                ps = qps.tile([128, 512], F32, tag="qkvps")
                for ct in range(8):
                    nc.tensor.matmul(
                        ps, xts[ct][:, tt * 128:(tt + 1) * 128], wv_sb[:, ct, :],
                        start=(ct == 0), stop=(ct == 7))
                nc.vector.tensor_tensor(
                    vbuf[:, tt, :, :],
                    ps.rearrange("p (h d) -> p h d", d=64),
                    bv_sb.rearrange("p (h d) -> p h d", d=64), ADD)

        # ---------------- attention + projection ----------------
        with tc.tile_pool(name="attnsb", bufs=1) as asb, \
             tc.tile_pool(name="utp", bufs=4) as utp, \
             tc.tile_pool(name="otp", bufs=1) as otp, \
             tc.tile_pool(name="nrm", bufs=2) as nrm, \
             tc.tile_pool(name="dpool", bufs=2, space="DRAM") as dpool:
            wp_sb = asb.tile([128, 16, 1024], BF16, tag="wp")
            nc.sync.dma_start(wp_sb[0:64], wp[:])
            nc.sync.dma_start(wp_sb[64:128], wp[:])
            # paired ot tiles: head 2hp at partitions 0:64, head 2hp+1 at 64:128
            ots = [otp.tile([128, 2048], BF16, tag=f"ot{hp}", name=f"ot{hp}")
                   for hp in range(4)]

            with tc.tile_pool(name="spool", bufs=2, space="PSUM") as spx, \
                 tc.tile_pool(name="opool", bufs=1, space="PSUM") as opx, \
                 tc.tile_pool(name="rpool", bufs=1, space="PSUM") as rpx, \
                 tc.tile_pool(name="ypool", bufs=2, space="PSUM") as ypx:

                def s_exp(hp, ic, j):
                    # heads 2hp (partitions 0:64) and 2hp+1 (64:128) of qk
                    # tile hp: row-packed matmuls run concurrently in the PE
                    # array and write separate psum banks of one tile.
                    kt = qk[hp]
                    qt = qk[4 + hp]
                    jsl = slice(j * 128, (j + 1) * 128)
                    isl = slice(ic * 512, (ic + 1) * 512)
                    sp = spx.tile([128, 1024], F32, tag="sp", name="sp")
                    nc.tensor.matmul(sp[:, 0:512], kt[0:64, jsl], qt[0:64, isl],
                                     start=True, stop=True)
                    nc.tensor.matmul(sp[:, 512:1024], kt[64:128, jsl],
                                     qt[64:128, isl], start=True, stop=True)
                    ut = utp.tile([128, 1024], BF16, tag="ut", name="ut")
                    nc.scalar.activation(ut, sp, EXP, scale=0.125)
                    return ut

                proj_state = {}

                def proj_mm(hp, q2, u):
                    # row-packed pair: head A weights at array rows 0:64,
                    # head B at 64:128; separate psum tiles.
                    if "ypA" not in proj_state:
                        proj_state["ypA"] = ypx.tile([128, 512], F32,
                                                     tag="yp", name="ypA")
                        proj_state["ypB"] = ypx.tile([128, 512], F32,
                                                     tag="yp", name="ypB")
                    ypA, ypB = proj_state["ypA"], proj_state["ypB"]
                    otr = ots[hp].rearrange("d (t u) -> d u t", u=16)
                    csl = slice(q2 * 512, (q2 + 1) * 512)
                    nc.tensor.matmul(ypA, otr[0:64, u, :], wp_sb[0:64, u, csl],
                                     start=(u == 0), stop=(u == 15))
                    nc.tensor.matmul(ypB, otr[64:128, u, :],
                                     wp_sb[64:128, u, csl],
                                     start=(u == 0), stop=(u == 15))
                    if u == 15:
                        for h, yp in ((2 * hp, ypA), (2 * hp + 1, ypB)):
                            ysb = yps.tile([128, 512], F32, tag="ysb",
                                           name="ysb")
                            nc.vector.tensor_tensor(ysb, yp, pb_sb[:, csl], ADD)
                            nc.sync.dma_start(y[:, h, csl], ysb)
                        proj_state.clear()

                # dense warm-up bridge across the QKV->attention pool boundary
                # so HAM never re-throttles the PE clock (inputs resident,
                # result unused).
                wrm = ypx.tile([128, 512], F32, tag="yp", name="wrm")
                for wi in range(18):
                    nc.tensor.matmul(
                        wrm, qk[3][0:64, 0:128], qk[3][0:64, 0:512],
                        start=(wi == 0), stop=(wi == 17))

                proj_q = []
                seq = [(hp, ic, j) for hp in range(4) for ic in range(4)
                       for j in range(16)]
                pend = s_exp(*seq[0])
                optile = rsps = None
                for idx, (hp, ic, j) in enumerate(seq):
                    hA, hB = 2 * hp, 2 * hp + 1
                    if j == 0:
                        optile = opx.tile([128, 512], F32, tag="op", name="op")
                        rsps = rpx.tile([33, 512], F32, tag="rs", name="rs")
                    nxt = s_exp(*seq[idx + 1]) if idx + 1 < len(seq) else None
                    # col-packed V pair: head A -> partitions 0:64, head B ->
                    # partitions 64:128 of one psum tile, concurrent in array
                    nc.tensor.matmul(optile[0:64, :], vbuf[:, j, hA, :],
                                     pend[:, 0:512],
                                     start=(j == 0), stop=(j == 15))
                    nc.tensor.matmul(optile[64:128, :], vbuf[:, j, hB, :],
                                     pend[:, 512:1024],
                                     start=(j == 0), stop=(j == 15),
                                     tile_position=(0, 64))
                    # softmax denominators: col-packed M=1 ones-matmuls
                    nc.tensor.matmul(rsps[0:1, :], ones128_sb, pend[:, 0:512],
                                     start=(j == 0), stop=(j == 15))
                    nc.tensor.matmul(rsps[32:33, :], ones128_sb,
                                     pend[:, 512:1024],
                                     start=(j == 0), stop=(j == 15),
                                     tile_position=(0, 32))
                    pend = nxt
                    # keep the PE array dense with projection matmuls of
                    # finished head pairs
                    if proj_q:
                        proj_mm(*proj_q.pop(0))
                    if j == 15:
                        # free psum right away, normalize both heads at once:
                        # denominators for A and B -> DRAM as one [1024] row,
                        # reciprocal at 8 els/lane, broadcast back into the
                        # matching partition halves.
                        osb = nrm.tile([128, 512], F32, tag="osb", name="osb")
                        nc.vector.tensor_copy(osb, optile)
                        rsb = nrm.tile([33, 512], F32, tag="rsb", name="rsb")
                        nc.vector.tensor_copy(rsb, rsps)
                        scr1 = dpool.tile([1024], F32, tag="scr1", name="scr1")
                        nc.sync.dma_start(
                            scr1.rearrange("(r f) -> r f", r=2), rsb[0:33:32, :])
                        rst = nrm.tile([128, 8], F32, tag="rst", name="rst")
                        nc.sync.dma_start(rst, scr1.rearrange("(p f) -> p f", f=8))
                        nc.vector.reciprocal(rst, rst)
                        scr2 = dpool.tile([1024], F32, tag="scr2", name="scr2")
                        nc.sync.dma_start(scr2.rearrange("(p f) -> p f", f=8), rst)
                        bcsb = nrm.tile([128, 512], F32, tag="bcsb", name="bcsb")
                        nc.sync.dma_start(
                            bcsb[0:64, :], scr2[None, 0:512].to_broadcast((64, 512)))
                        nc.sync.dma_start(
                            bcsb[64:128, :],
                            scr2[None, 512:1024].to_broadcast((64, 512)))
                        nc.vector.tensor_tensor(
                            ots[hp][:, ic * 512:(ic + 1) * 512], osb, bcsb, MULT)
                        if ic == 3:
                            proj_q.extend([(hp, q2, u) for q2 in range(2)
                                           for u in range(16)])
                # drain remaining projection matmuls (last head pair)
                while proj_q:
                    proj_mm(*proj_q.pop(0))

    nc.compile()
    return nc


def _in_maps(x, w_weight, w_bias, proj_weight, proj_bias):
    x = np.ascontiguousarray(x, np.float32)
    w_weight = np.ascontiguousarray(w_weight, np.float32)
    w_bias = np.ascontiguousarray(w_bias, np.float32)
    proj_weight = np.ascontiguousarray(proj_weight, np.float32)
    proj_bias = np.ascontiguousarray(proj_bias, np.float32)

    wpT = np.ascontiguousarray(
        proj_weight.T.reshape(16, 64, 1024).transpose(1, 0, 2).astype(ml_dtypes.bfloat16))
    pbr = np.ascontiguousarray(np.tile(proj_bias[None], (128, 1)))
    vones = np.ones((128, 1), dtype=ml_dtypes.bfloat16)
    ones1 = np.ones((1, 64), np.float32)

    maps = []
    for c in range(NCORES):
        b = c // 2
        h0 = (c % 2) * HPC
        xTc = np.ascontiguousarray(
            x[b].T.reshape(8, 128, 2048).transpose(1, 0, 2))
        wk = w_weight[h0 * 64: h0 * 64 + 512]
        wq = w_weight[1024 + h0 * 64: 1024 + h0 * 64 + 512]
        wv = w_weight[2048 + h0 * 64: 2048 + h0 * 64 + 512]
        wqkvT = np.concatenate([wk.T, wq.T, wv.T], axis=1)  # [1024, 1536]
        wqkvT = np.ascontiguousarray(
            wqkvT.reshape(8, 128, 1536).transpose(1, 0, 2))
        bk = w_bias[h0 * 64: h0 * 64 + 512]
        bq = w_bias[1024 + h0 * 64: 1024 + h0 * 64 + 512]
        bvc = w_bias[2048 + h0 * 64: 2048 + h0 * 64 + 512]
        bqkc = np.ascontiguousarray(
            np.concatenate([bk.reshape(4, 128).T, bq.reshape(4, 128).T], axis=1))
        bvr = np.ascontiguousarray(np.tile(bvc[None], (128, 1)))
        maps.append({
            "xT": xTc, "wqkv": wqkvT, "bqk": bqkc, "bv": bvr,
            "wp": wpT, "pb": pbr, "vones": vones, "ones1": ones1,
        })
    return maps


def _install_ntff_hook():
    """Register the axon NTFF profiling hook (missing antenv.axon_hooks shim)."""
    import contextlib
    import ctypes
    import types

    if "antenv.axon_hooks" in sys.modules:
        return
    import antenv
    so_path = "/opt/axon/libaxon_pjrt.so"
    try:
        lib = ctypes.CDLL(so_path)
    except OSError:
        return
    if not hasattr(lib, "axon_start_nrt_profile"):
        return
    lib.axon_start_nrt_profile.argtypes = [ctypes.POINTER(ctypes.c_int64),
                                           ctypes.c_size_t]
    lib.axon_start_nrt_profile.restype = ctypes.c_int64
    lib.axon_stop_nrt_profile.argtypes = [ctypes.c_char_p]
    lib.axon_stop_nrt_profile.restype = ctypes.c_int64

    @contextlib.contextmanager
    def _hook(output_dir, device_ids):
        import jax
        jax.devices()
        if device_ids:
            ids = (ctypes.c_int64 * len(device_ids))(*device_ids)
            rc = lib.axon_start_nrt_profile(ids, len(device_ids))
        else:
            rc = lib.axon_start_nrt_profile(None, 0)
        if rc != 0:
            raise RuntimeError(f"axon_start_nrt_profile rc={rc}")
        try:
            yield
        finally:
            n = lib.axon_stop_nrt_profile(str(output_dir).encode())
            print(f"profile: {n} file(s) written to {output_dir}", file=sys.stderr)

    mod = types.ModuleType("antenv.axon_hooks")
    mod.get_axon_ntff_profile_hook = lambda: _hook
    mod.set_axon_ntff_profile_hook = lambda h: None
    sys.modules["antenv.axon_hooks"] = mod
    antenv.axon_hooks = mod


def _run(x, w_weight, w_bias, proj_weight, proj_bias, trace=False):
    from concourse.bass_utils import run_bass_kernel_spmd

    if trace:
        _install_ntff_hook()

    if "nc" not in _CACHE:
        _CACHE["nc"] = _build()
    nc = _CACHE["nc"]
    maps = _in_maps(x, w_weight, w_bias, proj_weight, proj_bias)
    res = run_bass_kernel_spmd(nc, maps, core_ids=list(range(NCORES)), trace=trace)
    out = np.zeros((B, T, C), np.float32)
    for c in range(NCORES):
        yc = res.results[c]["y"]  # [128, 8, 1024]
        b = c // 2
        h0 = (c % 2) * HPC
        for j in range(HPC):
            out[b, (h0 + j) * 128:(h0 + j + 1) * 128, :] = yc[:, j, :]
    return out, res.exec_time_ns


def kernel(x, w_weight, w_bias, proj_weight, proj_bias):
    out, _ = _run(x, w_weight, w_bias, proj_weight, proj_bias, trace=False)
    return out


def kernel_with_time(x, w_weight, w_bias, proj_weight, proj_bias):
    return _run(x, w_weight, w_bias, proj_weight, proj_bias, trace=True)


# revision 20
# speedup vs baseline: 1.1323x; 1.1323x over previous
"""Fused attention kernel for Trainium2, 8 NeuronCores.

Problem: B=4, T=2048, C=1024, nh=16, hs=64, fused QKV (chunk order k,q,v),
softmax attention, then (faithful reference bug) reshape (B,nh,T,hs)->(B,T,C)
directly before the output projection.

Key structural fact: with the buggy reshape, head h's attention output
occupies exactly rows [h*128, (h+1)*128) of the reshaped (T, C) matrix
(row tau = h*128 + t//16, col = (t%16)*64 + d). So everything after the
QKV projection is fully independent per (batch, head) pair; the output
projection needs no cross-head reduction.

Sharding: 8 cores = 4 batches x 2 head-groups (8 heads each). Each core
computes its batch's QKV slice and its 8 heads end-to-end. No collectives.
"""

import sys

import numpy as np

sys.path.insert(0, "/opt/trn_rl_repo")

import ml_dtypes  # noqa: E402

B, T, C = 4, 2048, 1024
NH, HS = 16, 64
NCORES = 8
HPC = 8  # heads per core

_CACHE = {}


def _build():
    from contextlib import ExitStack

    import concourse.bass as bass  # noqa: F401
    import concourse.mybir as mybir
    from concourse import bacc, tile

    F32 = mybir.dt.float32
    F32R = mybir.dt.float32r
    BF16 = mybir.dt.bfloat16
    ADD = mybir.AluOpType.add
    MULT = mybir.AluOpType.mult
    EXP = mybir.ActivationFunctionType.Exp

    nc = bacc.Bacc()
    xT = nc.dram_tensor("xT", [128, 8, 2048], F32R, kind="ExternalInput")
    wqkv = nc.dram_tensor("wqkv", [128, 8, 1536], F32R, kind="ExternalInput")
    bqk = nc.dram_tensor("bqk", [128, 8], F32, kind="ExternalInput")
    bv = nc.dram_tensor("bv", [128, 512], F32, kind="ExternalInput")
    wp = nc.dram_tensor("wp", [64, 16, 1024], BF16, kind="ExternalInput")
    pb = nc.dram_tensor("pb", [128, 1024], F32, kind="ExternalInput")
    vones = nc.dram_tensor("vones", [128, 1], BF16, kind="ExternalInput")
    ones1 = nc.dram_tensor("ones1", [1, 64], F32R, kind="ExternalInput")
    y = nc.dram_tensor("y", [128, 8, 1024], F32, kind="ExternalOutput")

    with tile.TileContext(nc) as tc, ExitStack() as ctx:
        persist = ctx.enter_context(tc.tile_pool(name="persist", bufs=1))
        yps = ctx.enter_context(tc.tile_pool(name="ysb", bufs=2))

        scratch = persist.tile([128, 4], F32, tag="scratch")
        bqk_sb = persist.tile([128, 8], F32, tag="bqk")
        nc.sync.dma_start(bqk_sb, bqk[:])
        nc.vector.tensor_copy(scratch[:, 0:1], bqk_sb[:, 0:1])
        pb_sb = persist.tile([128, 1024], F32, tag="pb")
        nc.sync.dma_start(pb_sb, pb[:])
        nc.vector.tensor_copy(scratch[:, 1:2], pb_sb[:, 0:1])
        ones1_sb = persist.tile([1, 64], F32R, tag="ones1")
        nc.sync.dma_start(ones1_sb, ones1[:])
        vbuf = persist.tile([128, 16, HPC, 64], BF16, tag="vbuf")
        ones128_sb = persist.tile([128, 1], BF16, tag="ones128")
        nc.sync.dma_start(ones128_sb, vones[:])
        nc.vector.tensor_copy(scratch[:, 2:3], ones128_sb)
        # K^T rows in tiles 0-3 (d on partitions, t free), Q^T rows in 4-7
        qk = [persist.tile([128, 2048], F32R, tag=f"qk{mt}", name=f"qk{mt}")
              for mt in range(8)]

        # ---------------- QKV phase ----------------
        with tc.tile_pool(name="qkvsb", bufs=1) as qsb, \
             tc.tile_pool(name="wstream", bufs=2) as wsp, \
             tc.tile_pool(name="qkvps", bufs=2, space="PSUM") as qps:
            bv_sb = qsb.tile([128, 512], F32, tag="bv")
            nc.sync.dma_start(bv_sb, bv[:])
            nc.vector.tensor_copy(scratch[:, 3:4], bv_sb[:, 0:1])
            xts = [qsb.tile([128, 2048], F32R, tag=f"xt{ct}", name=f"xt{ct}")
                   for ct in range(8)]
            for q in range(4):
                for ct in range(8):
                    nc.sync.dma_start(xts[ct][:, q * 512:(q + 1) * 512],
                                      xT[:, ct, q * 512:(q + 1) * 512])

            for mt in range(8):
                wt = wsp.tile([128, 8, 128], F32R, tag="wt", name="wt")
                nc.sync.dma_start(wt, wqkv[:, :, mt * 128:(mt + 1) * 128])
                for icx in range(4):
                    ps = qps.tile([128, 512], F32, tag="qkvps")
                    for ct in range(8):
                        nc.tensor.matmul(
                            ps, wt[:, ct, :], xts[ct][:, icx * 512:(icx + 1) * 512],
                            start=(ct == 0), stop=(ct == 7))
                    nc.vector.tensor_tensor(
                        qk[mt][:, icx * 512:(icx + 1) * 512], ps,
                        bqk_sb[:, mt:mt + 1].to_broadcast((128, 512)), ADD)

            wv_sb = qsb.tile([128, 8, 512], F32R, tag="wv")
            nc.sync.dma_start(wv_sb, wqkv[:, :, 1024:1536])
            for tt in range(16):
                ps = qps.tile([128, 512], F32, tag="qkvps")
                for ct in range(8):
                    nc.tensor.matmul(
                        ps, xts[ct][:, tt * 128:(tt + 1) * 128], wv_sb[:, ct, :],
                        start=(ct == 0), stop=(ct == 7))
                nc.vector.tensor_tensor(
                    vbuf[:, tt, :, :],
                    ps.rearrange("p (h d) -> p h d", d=64),
                    bv_sb.rearrange("p (h d) -> p h d", d=64), ADD)

        # ---------------- attention + projection ----------------
        with tc.tile_pool(name="attnsb", bufs=1) as asb, \
             tc.tile_pool(name="utp", bufs=4) as utp, \
             tc.tile_pool(name="otp", bufs=1) as otp, \
             tc.tile_pool(name="nrm", bufs=2) as nrm, \
             tc.tile_pool(name="dpool", bufs=2, space="DRAM") as dpool:
            wp_sb = asb.tile([128, 16, 1024], BF16, tag="wp")
            nc.sync.dma_start(wp_sb[0:64], wp[:])
            nc.sync.dma_start(wp_sb[64:128], wp[:])
            # paired ot tiles: head 2hp at partitions 0:64, head 2hp+1 at 64:128
            ots = [otp.tile([128, 2048], BF16, tag=f"ot{hp}", name=f"ot{hp}")
                   for hp in range(4)]

            with tc.tile_pool(name="spool", bufs=2, space="PSUM") as spx, \
                 tc.tile_pool(name="opool", bufs=1, space="PSUM") as opx, \
                 tc.tile_pool(name="rpool", bufs=1, space="PSUM") as rpx, \
                 tc.tile_pool(name="ypool", bufs=2, space="PSUM") as ypx:

                def s_exp(hp, ic, j):
                    # heads 2hp (partitions 0:64) and 2hp+1 (64:128) of qk
                    # tile hp: row-packed matmuls run concurrently in the PE
                    # array and write separate psum banks of one tile.
                    kt = qk[hp]
                    qt = qk[4 + hp]
                    jsl = slice(j * 128, (j + 1) * 128)
                    isl = slice(ic * 512, (ic + 1) * 512)
                    sp = spx.tile([128, 1024], F32, tag="sp", name="sp")
                    nc.tensor.matmul(sp[:, 0:512], kt[0:64, jsl], qt[0:64, isl],
                                     start=True, stop=True)
                    nc.tensor.matmul(sp[:, 512:1024], kt[64:128, jsl],
                                     qt[64:128, isl], start=True, stop=True)
                    ut = utp.tile([128, 1024], BF16, tag="ut", name="ut")
                    nc.scalar.activation(ut, sp, EXP, scale=0.125)
                    return ut

                proj_state = {}

                def proj_mm(hp, q2, u):
                    # row-packed pair: head A weights at array rows 0:64,
                    # head B at 64:128; separate psum tiles.
                    if "ypA" not in proj_state:
                        proj_state["ypA"] = ypx.tile([128, 512], F32,
                                                     tag="yp", name="ypA")
                        proj_state["ypB"] = ypx.tile([128, 512], F32,
                                                     tag="yp", name="ypB")
                    ypA, ypB = proj_state["ypA"], proj_state["ypB"]
                    otr = ots[hp].rearrange("d (t u) -> d u t", u=16)
                    csl = slice(q2 * 512, (q2 + 1) * 512)
                    nc.tensor.matmul(ypA, otr[0:64, u, :], wp_sb[0:64, u, csl],
                                     start=(u == 0), stop=(u == 15))
                    nc.tensor.matmul(ypB, otr[64:128, u, :],
                                     wp_sb[64:128, u, csl],
                                     start=(u == 0), stop=(u == 15))
                    if u == 15:
                        for h, yp in ((2 * hp, ypA), (2 * hp + 1, ypB)):
                            ysb = yps.tile([128, 512], F32, tag="ysb",
                                           name="ysb")
                            nc.vector.tensor_tensor(ysb, yp, pb_sb[:, csl], ADD)
                            nc.sync.dma_start(y[:, h, csl], ysb)
                        proj_state.clear()

                # dense warm-up bridge across the QKV->attention pool boundary
                # so HAM never re-throttles the PE clock (inputs resident,
                # result unused).
                wrm = ypx.tile([128, 512], F32, tag="yp", name="wrm")
                for wi in range(18):
                    nc.tensor.matmul(
                        wrm, qk[3][0:64, 0:128], qk[3][0:64, 0:512],
                        start=(wi == 0), stop=(wi == 17))

                proj_q = []
                seq = [(hp, ic, j) for hp in range(4) for ic in range(4)
                       for j in range(16)]
                pend = s_exp(*seq[0])
                optile = rsps = None
                for idx, (hp, ic, j) in enumerate(seq):
                    hA, hB = 2 * hp, 2 * hp + 1
                    if j == 0:
                        optile = opx.tile([128, 512], F32, tag="op", name="op")
                        rsps = rpx.tile([33, 512], F32, tag="rs", name="rs")
                    nxt = s_exp(*seq[idx + 1]) if idx + 1 < len(seq) else None
                    # col-packed V pair: head A -> partitions 0:64, head B ->
                    # partitions 64:128 of one psum tile, concurrent in array
                    nc.tensor.matmul(optile[0:64, :], vbuf[:, j, hA, :],
                                     pend[:, 0:512],
                                     start=(j == 0), stop=(j == 15))
                    nc.tensor.matmul(optile[64:128, :], vbuf[:, j, hB, :],
                                     pend[:, 512:1024],
                                     start=(j == 0), stop=(j == 15),
                                     tile_position=(0, 64))
                    # softmax denominators: col-packed M=1 ones-matmuls
                    nc.tensor.matmul(rsps[0:1, :], ones128_sb, pend[:, 0:512],
                                     start=(j == 0), stop=(j == 15))
                    nc.tensor.matmul(rsps[32:33, :], ones128_sb,
                                     pend[:, 512:1024],
                                     start=(j == 0), stop=(j == 15),
                                     tile_position=(0, 32))
                    pend = nxt
                    # keep the PE array dense with projection matmuls of
                    # finished head pairs
                    if proj_q:
                        proj_mm(*proj_q.pop(0))
                    if j == 15:
                        # free psum right away, normalize both heads at once:
                        # denominators for A and B -> DRAM as one [1024] row,
                        # reciprocal at 8 els/lane, broadcast back into the
                        # matching partition halves.
                        osb = nrm.tile([128, 512], F32, tag="osb", name="osb")
                        nc.vector.tensor_copy(osb, optile)
                        rsb = nrm.tile([33, 512], F32, tag="rsb", name="rsb")
                        nc.vector.tensor_copy(rsb, rsps)
                        scr1 = dpool.tile([1024], F32, tag="scr1", name="scr1")
                        nc.sync.dma_start(
                            scr1.rearrange("(r f) -> r f", r=2), rsb[0:33:32, :])
                        rst = nrm.tile([128, 8], F32, tag="rst", name="rst")
                        nc.sync.dma_start(rst, scr1.rearrange("(p f) -> p f", f=8))
                        nc.vector.reciprocal(rst, rst)
                        scr2 = dpool.tile([1024], F32, tag="scr2", name="scr2")
                        nc.sync.dma_start(scr2.rearrange("(p f) -> p f", f=8), rst)
                        bcsb = nrm.tile([128, 512], F32, tag="bcsb", name="bcsb")
                        nc.sync.dma_start(
                            bcsb[0:64, :], scr2[None, 0:512].to_broadcast((64, 512)))
                        nc.sync.dma_start(
                            bcsb[64:128, :],
                            scr2[None, 512:1024].to_broadcast((64, 512)))
                        nc.vector.tensor_tensor(
                            ots[hp][:, ic * 512:(ic + 1) * 512], osb, bcsb, MULT)
                        if ic == 3:
                            proj_q.extend([(hp, q2, u) for q2 in range(2)
                                           for u in range(16)])
                # drain remaining projection matmuls (last head pair)
                while proj_q:
                    proj_mm(*proj_q.pop(0))

    nc.compile()
    return nc


def _in_maps(x, w_weight, w_bias, proj_weight, proj_bias):
    x = np.ascontiguousarray(x, np.float32)
    w_weight = np.ascontiguousarray(w_weight, np.float32)
    w_bias = np.ascontiguousarray(w_bias, np.float32)
    proj_weight = np.ascontiguousarray(proj_weight, np.float32)
    proj_bias = np.ascontiguousarray(proj_bias, np.float32)

    wpT = np.ascontiguousarray(
        proj_weight.T.reshape(16, 64, 1024).transpose(1, 0, 2).astype(ml_dtypes.bfloat16))
    pbr = np.ascontiguousarray(np.tile(proj_bias[None], (128, 1)))
    vones = np.ones((128, 1), dtype=ml_dtypes.bfloat16)
    ones1 = np.ones((1, 64), np.float32)

    maps = []
    for c in range(NCORES):
        b = c // 2
        h0 = (c % 2) * HPC
        xTc = np.ascontiguousarray(
            x[b].T.reshape(8, 128, 2048).transpose(1, 0, 2))
        wk = w_weight[h0 * 64: h0 * 64 + 512]
        wq = w_weight[1024 + h0 * 64: 1024 + h0 * 64 + 512]
        wv = w_weight[2048 + h0 * 64: 2048 + h0 * 64 + 512]
        wqkvT = np.concatenate([wk.T, wq.T, wv.T], axis=1)  # [1024, 1536]
        wqkvT = np.ascontiguousarray(
            wqkvT.reshape(8, 128, 1536).transpose(1, 0, 2))
        bk = w_bias[h0 * 64: h0 * 64 + 512]
        bq = w_bias[1024 + h0 * 64: 1024 + h0 * 64 + 512]
        bvc = w_bias[2048 + h0 * 64: 2048 + h0 * 64 + 512]
        bqkc = np.ascontiguousarray(
            np.concatenate([bk.reshape(4, 128).T, bq.reshape(4, 128).T], axis=1))
        bvr = np.ascontiguousarray(np.tile(bvc[None], (128, 1)))
        maps.append({
            "xT": xTc, "wqkv": wqkvT, "bqk": bqkc, "bv": bvr,
            "wp": wpT, "pb": pbr, "vones": vones, "ones1": ones1,
        })
    return maps


def _install_ntff_hook():
    """Register the axon NTFF profiling hook (missing antenv.axon_hooks shim)."""
    import contextlib
    import ctypes
    import types

    if "antenv.axon_hooks" in sys.modules:
        return
    import antenv
    so_path = "/opt/axon/libaxon_pjrt.so"
    try:
        lib = ctypes.CDLL(so_path)
    except OSError:
        return
    if not hasattr(lib, "axon_start_nrt_profile"):
        return
    lib.axon_start_nrt_profile.argtypes = [ctypes.POINTER(ctypes.c_int64),
                                           ctypes.c_size_t]
    lib.axon_start_nrt_profile.restype = ctypes.c_int64
    lib.axon_stop_nrt_profile.argtypes = [ctypes.c_char_p]
    lib.axon_stop_nrt_profile.restype = ctypes.c_int64

    @contextlib.contextmanager
    def _hook(output_dir, device_ids):
        import jax
        jax.devices()
        if device_ids:
            ids = (ctypes.c_int64 * len(device_ids))(*device_ids)
            rc = lib.axon_start_nrt_profile(ids, len(device_ids))
        else:
            rc = lib.axon_start_nrt_profile(None, 0)
        if rc != 0:
            raise RuntimeError(f"axon_start_nrt_profile rc={rc}")
        try:
            yield
        finally:
            n = lib.axon_stop_nrt_profile(str(output_dir).encode())
            print(f"profile: {n} file(s) written to {output_dir}", file=sys.stderr)

    mod = types.ModuleType("antenv.axon_hooks")
    mod.get_axon_ntff_profile_hook = lambda: _hook
    mod.set_axon_ntff_profile_hook = lambda h: None
    sys.modules["antenv.axon_hooks"] = mod
    antenv.axon_hooks = mod


def _run(x, w_weight, w_bias, proj_weight, proj_bias, trace=False):
    from concourse.bass_utils import run_bass_kernel_spmd

    if trace:
        _install_ntff_hook()

    if "nc" not in _CACHE:
        _CACHE["nc"] = _build()
    nc = _CACHE["nc"]
    maps = _in_maps(x, w_weight, w_bias, proj_weight, proj_bias)
    res = run_bass_kernel_spmd(nc, maps, core_ids=list(range(NCORES)), trace=trace)
    out = np.zeros((B, T, C), np.float32)
    for c in range(NCORES):
        yc = res.results[c]["y"]  # [128, 8, 1024]
        b = c // 2
        h0 = (c % 2) * HPC
        for j in range(HPC):
            out[b, (h0 + j) * 128:(h0 + j + 1) * 128, :] = yc[:, j, :]
    return out, res.exec_time_ns


def kernel(x, w_weight, w_bias, proj_weight, proj_bias):
    out, _ = _run(x, w_weight, w_bias, proj_weight, proj_bias, trace=False)
    return out


def kernel_with_time(x, w_weight, w_bias, proj_weight, proj_bias):
    return _run(x, w_weight, w_bias, proj_weight, proj_bias, trace=True)


# revision 21
# speedup vs baseline: 1.1500x; 1.0156x over previous
"""Fused attention kernel for Trainium2, 8 NeuronCores.

Problem: B=4, T=2048, C=1024, nh=16, hs=64, fused QKV (chunk order k,q,v),
softmax attention, then (faithful reference bug) reshape (B,nh,T,hs)->(B,T,C)
directly before the output projection.

Key structural fact: with the buggy reshape, head h's attention output
occupies exactly rows [h*128, (h+1)*128) of the reshaped (T, C) matrix
(row tau = h*128 + t//16, col = (t%16)*64 + d). So everything after the
QKV projection is fully independent per (batch, head) pair; the output
projection needs no cross-head reduction.

Sharding: 8 cores = 4 batches x 2 head-groups (8 heads each). Each core
computes its batch's QKV slice and its 8 heads end-to-end. No collectives.
"""

import sys

import numpy as np

sys.path.insert(0, "/opt/trn_rl_repo")

import ml_dtypes  # noqa: E402

B, T, C = 4, 2048, 1024
NH, HS = 16, 64
NCORES = 8
HPC = 8  # heads per core

_CACHE = {}


def _build():
    from contextlib import ExitStack

    import concourse.bass as bass  # noqa: F401
    import concourse.mybir as mybir
    from concourse import bacc, tile

    F32 = mybir.dt.float32
    F32R = mybir.dt.float32r
    BF16 = mybir.dt.bfloat16
    ADD = mybir.AluOpType.add
    MULT = mybir.AluOpType.mult
    EXP = mybir.ActivationFunctionType.Exp

    nc = bacc.Bacc()
    xT = nc.dram_tensor("xT", [128, 8, 2048], BF16, kind="ExternalInput")
    wqkv = nc.dram_tensor("wqkv", [128, 8, 1536], BF16, kind="ExternalInput")
    bqk = nc.dram_tensor("bqk", [128, 8], F32, kind="ExternalInput")
    bv = nc.dram_tensor("bv", [128, 512], F32, kind="ExternalInput")
    wp = nc.dram_tensor("wp", [64, 16, 1024], BF16, kind="ExternalInput")
    pb = nc.dram_tensor("pb", [128, 1024], F32, kind="ExternalInput")
    vones = nc.dram_tensor("vones", [128, 1], BF16, kind="ExternalInput")
    ones1 = nc.dram_tensor("ones1", [1, 64], F32R, kind="ExternalInput")
    y = nc.dram_tensor("y", [128, 8, 1024], F32, kind="ExternalOutput")

    with tile.TileContext(nc) as tc, ExitStack() as ctx:
        persist = ctx.enter_context(tc.tile_pool(name="persist", bufs=1))
        yps = ctx.enter_context(tc.tile_pool(name="ysb", bufs=2))

        scratch = persist.tile([128, 4], F32, tag="scratch")
        bqk_sb = persist.tile([128, 8], F32, tag="bqk")
        nc.sync.dma_start(bqk_sb, bqk[:])
        nc.vector.tensor_copy(scratch[:, 0:1], bqk_sb[:, 0:1])
        pb_sb = persist.tile([128, 1024], F32, tag="pb")
        nc.sync.dma_start(pb_sb, pb[:])
        nc.vector.tensor_copy(scratch[:, 1:2], pb_sb[:, 0:1])
        ones1_sb = persist.tile([1, 64], F32R, tag="ones1")
        nc.sync.dma_start(ones1_sb, ones1[:])
        vbuf = persist.tile([128, 16, HPC, 64], BF16, tag="vbuf")
        ones128_sb = persist.tile([128, 1], BF16, tag="ones128")
        nc.sync.dma_start(ones128_sb, vones[:])
        nc.vector.tensor_copy(scratch[:, 2:3], ones128_sb)
        # K^T rows in tiles 0-3 (d on partitions, t free), Q^T rows in 4-7
        qk = [persist.tile([128, 2048], F32R, tag=f"qk{mt}", name=f"qk{mt}")
              for mt in range(8)]

        # ---------------- QKV phase ----------------
        with tc.tile_pool(name="qkvsb", bufs=1) as qsb, \
             tc.tile_pool(name="wstream", bufs=2) as wsp, \
             tc.tile_pool(name="qkvps", bufs=2, space="PSUM") as qps:
            bv_sb = qsb.tile([128, 512], F32, tag="bv")
            nc.sync.dma_start(bv_sb, bv[:])
            nc.vector.tensor_copy(scratch[:, 3:4], bv_sb[:, 0:1])
            xts = [qsb.tile([128, 2048], BF16, tag=f"xt{ct}", name=f"xt{ct}")
                   for ct in range(8)]
            for q in range(4):
                for ct in range(8):
                    nc.sync.dma_start(xts[ct][:, q * 512:(q + 1) * 512],
                                      xT[:, ct, q * 512:(q + 1) * 512])

            for mt in range(8):
                wt = wsp.tile([128, 8, 128], BF16, tag="wt", name="wt")
                nc.sync.dma_start(wt, wqkv[:, :, mt * 128:(mt + 1) * 128])
                for icx in range(4):
                    ps = qps.tile([128, 512], F32, tag="qkvps")
                    for ct in range(8):
                        nc.tensor.matmul(
                            ps, wt[:, ct, :], xts[ct][:, icx * 512:(icx + 1) * 512],
                            start=(ct == 0), stop=(ct == 7))
                    nc.vector.tensor_tensor(
                        qk[mt][:, icx * 512:(icx + 1) * 512], ps,
                        bqk_sb[:, mt:mt + 1].to_broadcast((128, 512)), ADD)

            wv_sb = qsb.tile([128, 8, 512], BF16, tag="wv")
            nc.sync.dma_start(wv_sb, wqkv[:, :, 1024:1536])
            for tt in range(16):
                ps = qps.tile([128, 512], F32, tag="qkvps")
                for ct in range(8):
                    nc.tensor.matmul(
                        ps, xts[ct][:, tt * 128:(tt + 1) * 128], wv_sb[:, ct, :],
                        start=(ct == 0), stop=(ct == 7))
                nc.vector.tensor_tensor(
                    vbuf[:, tt, :, :],
                    ps.rearrange("p (h d) -> p h d", d=64),
                    bv_sb.rearrange("p (h d) -> p h d", d=64), ADD)

        # ---------------- attention + projection ----------------
        with tc.tile_pool(name="attnsb", bufs=1) as asb, \
             tc.tile_pool(name="utp", bufs=4) as utp, \
             tc.tile_pool(name="otp", bufs=1) as otp, \
             tc.tile_pool(name="nrm", bufs=2) as nrm, \
             tc.tile_pool(name="dpool", bufs=2, space="DRAM") as dpool:
            wp_sb = asb.tile([128, 16, 1024], BF16, tag="wp")
            nc.sync.dma_start(wp_sb[0:64], wp[:])
            nc.sync.dma_start(wp_sb[64:128], wp[:])
            # paired ot tiles: head 2hp at partitions 0:64, head 2hp+1 at 64:128
            ots = [otp.tile([128, 2048], BF16, tag=f"ot{hp}", name=f"ot{hp}")
                   for hp in range(4)]

            with tc.tile_pool(name="spool", bufs=2, space="PSUM") as spx, \
                 tc.tile_pool(name="opool", bufs=1, space="PSUM") as opx, \
                 tc.tile_pool(name="rpool", bufs=1, space="PSUM") as rpx, \
                 tc.tile_pool(name="ypool", bufs=2, space="PSUM") as ypx:

                def s_exp(hp, ic, j):
                    # heads 2hp (partitions 0:64) and 2hp+1 (64:128) of qk
                    # tile hp: row-packed matmuls run concurrently in the PE
                    # array and write separate psum banks of one tile.
                    kt = qk[hp]
                    qt = qk[4 + hp]
                    jsl = slice(j * 128, (j + 1) * 128)
                    isl = slice(ic * 512, (ic + 1) * 512)
                    sp = spx.tile([128, 1024], F32, tag="sp", name="sp")
                    nc.tensor.matmul(sp[:, 0:512], kt[0:64, jsl], qt[0:64, isl],
                                     start=True, stop=True)
                    nc.tensor.matmul(sp[:, 512:1024], kt[64:128, jsl],
                                     qt[64:128, isl], start=True, stop=True)
                    ut = utp.tile([128, 1024], BF16, tag="ut", name="ut")
                    nc.scalar.activation(ut, sp, EXP, scale=0.125)
                    return ut

                proj_state = {}

                def proj_mm(hp, q2, u):
                    # row-packed pair: head A weights at array rows 0:64,
                    # head B at 64:128; separate psum tiles.
                    if "ypA" not in proj_state:
                        proj_state["ypA"] = ypx.tile([128, 512], F32,
                                                     tag="yp", name="ypA")
                        proj_state["ypB"] = ypx.tile([128, 512], F32,
                                                     tag="yp", name="ypB")
                    ypA, ypB = proj_state["ypA"], proj_state["ypB"]
                    otr = ots[hp].rearrange("d (t u) -> d u t", u=16)
                    csl = slice(q2 * 512, (q2 + 1) * 512)
                    nc.tensor.matmul(ypA, otr[0:64, u, :], wp_sb[0:64, u, csl],
                                     start=(u == 0), stop=(u == 15))
                    nc.tensor.matmul(ypB, otr[64:128, u, :],
                                     wp_sb[64:128, u, csl],
                                     start=(u == 0), stop=(u == 15))
                    if u == 15:
                        for h, yp in ((2 * hp, ypA), (2 * hp + 1, ypB)):
                            ysb = yps.tile([128, 512], F32, tag="ysb",
                                           name="ysb")
                            nc.vector.tensor_tensor(ysb, yp, pb_sb[:, csl], ADD)
                            nc.sync.dma_start(y[:, h, csl], ysb)
                        proj_state.clear()

                # dense warm-up bridge across the QKV->attention pool boundary
                # so HAM never re-throttles the PE clock (inputs resident,
                # result unused).
                wrm = ypx.tile([128, 512], F32, tag="yp", name="wrm")
                for wi in range(18):
                    nc.tensor.matmul(
                        wrm, qk[3][0:64, 0:128], qk[3][0:64, 0:512],
                        start=(wi == 0), stop=(wi == 17))

                proj_q = []
                seq = [(hp, ic, j) for hp in range(4) for ic in range(4)
                       for j in range(16)]
                pend = s_exp(*seq[0])
                optile = rsps = None
                for idx, (hp, ic, j) in enumerate(seq):
                    hA, hB = 2 * hp, 2 * hp + 1
                    if j == 0:
                        optile = opx.tile([128, 512], F32, tag="op", name="op")
                        rsps = rpx.tile([33, 512], F32, tag="rs", name="rs")
                    nxt = s_exp(*seq[idx + 1]) if idx + 1 < len(seq) else None
                    # col-packed V pair: head A -> partitions 0:64, head B ->
                    # partitions 64:128 of one psum tile, concurrent in array
                    nc.tensor.matmul(optile[0:64, :], vbuf[:, j, hA, :],
                                     pend[:, 0:512],
                                     start=(j == 0), stop=(j == 15))
                    nc.tensor.matmul(optile[64:128, :], vbuf[:, j, hB, :],
                                     pend[:, 512:1024],
                                     start=(j == 0), stop=(j == 15),
                                     tile_position=(0, 64))
                    # softmax denominators: col-packed M=1 ones-matmuls
                    nc.tensor.matmul(rsps[0:1, :], ones128_sb, pend[:, 0:512],
                                     start=(j == 0), stop=(j == 15))
                    nc.tensor.matmul(rsps[32:33, :], ones128_sb,
                                     pend[:, 512:1024],
                                     start=(j == 0), stop=(j == 15),
                                     tile_position=(0, 32))
                    pend = nxt
                    # keep the PE array dense with projection matmuls of
                    # finished head pairs
                    if proj_q:
                        proj_mm(*proj_q.pop(0))
                    if j == 15:
                        # free psum right away, normalize both heads at once:
                        # denominators for A and B -> DRAM as one [1024] row,
                        # reciprocal at 8 els/lane, broadcast back into the
                        # matching partition halves.
                        osb = nrm.tile([128, 512], F32, tag="osb", name="osb")
                        nc.vector.tensor_copy(osb, optile)
                        rsb = nrm.tile([33, 512], F32, tag="rsb", name="rsb")
                        nc.vector.tensor_copy(rsb, rsps)
                        scr1 = dpool.tile([1024], F32, tag="scr1", name="scr1")
                        nc.sync.dma_start(
                            scr1.rearrange("(r f) -> r f", r=2), rsb[0:33:32, :])
                        rst = nrm.tile([128, 8], F32, tag="rst", name="rst")
                        nc.sync.dma_start(rst, scr1.rearrange("(p f) -> p f", f=8))
                        nc.vector.reciprocal(rst, rst)
                        scr2 = dpool.tile([1024], F32, tag="scr2", name="scr2")
                        nc.sync.dma_start(scr2.rearrange("(p f) -> p f", f=8), rst)
                        bcsb = nrm.tile([128, 512], F32, tag="bcsb", name="bcsb")
                        nc.sync.dma_start(
                            bcsb[0:64, :], scr2[None, 0:512].to_broadcast((64, 512)))
                        nc.sync.dma_start(
                            bcsb[64:128, :],
                            scr2[None, 512:1024].to_broadcast((64, 512)))
                        nc.vector.tensor_tensor(
                            ots[hp][:, ic * 512:(ic + 1) * 512], osb, bcsb, MULT)
                        if ic == 3:
                            proj_q.extend([(hp, q2, u) for q2 in range(2)
                                           for u in range(16)])
                # drain remaining projection matmuls (last head pair)
                while proj_q:
                    proj_mm(*proj_q.pop(0))

    nc.compile()
    return nc


def _in_maps(x, w_weight, w_bias, proj_weight, proj_bias):
    x = np.ascontiguousarray(x, np.float32)
    w_weight = np.ascontiguousarray(w_weight, np.float32)
    w_bias = np.ascontiguousarray(w_bias, np.float32)
    proj_weight = np.ascontiguousarray(proj_weight, np.float32)
    proj_bias = np.ascontiguousarray(proj_bias, np.float32)

    wpT = np.ascontiguousarray(
        proj_weight.T.reshape(16, 64, 1024).transpose(1, 0, 2).astype(ml_dtypes.bfloat16))
    pbr = np.ascontiguousarray(np.tile(proj_bias[None], (128, 1)))
    vones = np.ones((128, 1), dtype=ml_dtypes.bfloat16)
    ones1 = np.ones((1, 64), np.float32)

    maps = []
    for c in range(NCORES):
        b = c // 2
        h0 = (c % 2) * HPC
        xTc = np.ascontiguousarray(
            x[b].T.reshape(8, 128, 2048).transpose(1, 0, 2).astype(ml_dtypes.bfloat16))
        wk = w_weight[h0 * 64: h0 * 64 + 512]
        wq = w_weight[1024 + h0 * 64: 1024 + h0 * 64 + 512]
        wv = w_weight[2048 + h0 * 64: 2048 + h0 * 64 + 512]
        wqkvT = np.concatenate([wk.T, wq.T, wv.T], axis=1)  # [1024, 1536]
        wqkvT = np.ascontiguousarray(
            wqkvT.reshape(8, 128, 1536).transpose(1, 0, 2).astype(ml_dtypes.bfloat16))
        bk = w_bias[h0 * 64: h0 * 64 + 512]
        bq = w_bias[1024 + h0 * 64: 1024 + h0 * 64 + 512]
        bvc = w_bias[2048 + h0 * 64: 2048 + h0 * 64 + 512]
        bqkc = np.ascontiguousarray(
            np.concatenate([bk.reshape(4, 128).T, bq.reshape(4, 128).T], axis=1))
        bvr = np.ascontiguousarray(np.tile(bvc[None], (128, 1)))
        maps.append({
            "xT": xTc, "wqkv": wqkvT, "bqk": bqkc, "bv": bvr,
            "wp": wpT, "pb": pbr, "vones": vones, "ones1": ones1,
        })
    return maps


def _install_ntff_hook():
    """Register the axon NTFF profiling hook (missing antenv.axon_hooks shim)."""
    import contextlib
    import ctypes
    import types

    if "antenv.axon_hooks" in sys.modules:
        return
    import antenv
    so_path = "/opt/axon/libaxon_pjrt.so"
    try:
        lib = ctypes.CDLL(so_path)
    except OSError:
        return
    if not hasattr(lib, "axon_start_nrt_profile"):
        return
    lib.axon_start_nrt_profile.argtypes = [ctypes.POINTER(ctypes.c_int64),
                                           ctypes.c_size_t]
    lib.axon_start_nrt_profile.restype = ctypes.c_int64
    lib.axon_stop_nrt_profile.argtypes = [ctypes.c_char_p]
    lib.axon_stop_nrt_profile.restype = ctypes.c_int64

    @contextlib.contextmanager
    def _hook(output_dir, device_ids):
        import jax
        jax.devices()
        if device_ids:
            ids = (ctypes.c_int64 * len(device_ids))(*device_ids)
            rc = lib.axon_start_nrt_profile(ids, len(device_ids))
        else:
            rc = lib.axon_start_nrt_profile(None, 0)
        if rc != 0:
            raise RuntimeError(f"axon_start_nrt_profile rc={rc}")
        try:
            yield
        finally:
            n = lib.axon_stop_nrt_profile(str(output_dir).encode())
            print(f"profile: {n} file(s) written to {output_dir}", file=sys.stderr)

    mod = types.ModuleType("antenv.axon_hooks")
    mod.get_axon_ntff_profile_hook = lambda: _hook
    mod.set_axon_ntff_profile_hook = lambda h: None
    sys.modules["antenv.axon_hooks"] = mod
    antenv.axon_hooks = mod


def _run(x, w_weight, w_bias, proj_weight, proj_bias, trace=False):
    from concourse.bass_utils import run_bass_kernel_spmd

    if trace:
        _install_ntff_hook()

    if "nc" not in _CACHE:
        _CACHE["nc"] = _build()
    nc = _CACHE["nc"]
    maps = _in_maps(x, w_weight, w_bias, proj_weight, proj_bias)
    res = run_bass_kernel_spmd(nc, maps, core_ids=list(range(NCORES)), trace=trace)
    out = np.zeros((B, T, C), np.float32)
    for c in range(NCORES):
        yc = res.results[c]["y"]  # [128, 8, 1024]
        b = c // 2
        h0 = (c % 2) * HPC
        for j in range(HPC):
            out[b, (h0 + j) * 128:(h0 + j + 1) * 128, :] = yc[:, j, :]
    return out, res.exec_time_ns


def kernel(x, w_weight, w_bias, proj_weight, proj_bias):
    out, _ = _run(x, w_weight, w_bias, proj_weight, proj_bias, trace=False)
    return out


def kernel_with_time(x, w_weight, w_bias, proj_weight, proj_bias):
    return _run(x, w_weight, w_bias, proj_weight, proj_bias, trace=True)
